# revision 15
# baseline (speedup 1.0000x reference)
"""Trainium2 Bass kernel for per-node LocalConv1D (kernel_size=1).

out[b, o, n] = sum_h W[n, o, h] * x[b, h, n] + b[n, o]

Full shapes: x [16, 32, 50000] f32, W [50000, 32, 32] f32, b [50000, 32] f32,
out [16, 32, 50000] f32.

Sharding: node dim n split evenly across 8 NeuronCores (6250 nodes/core,
zero-padded to 6272 = 49*128 inside each shard). Fully independent per-node
32x32 matmuls -> no collectives.

Per-core strategy (memory-bound problem, ~52 MB of HBM traffic per core):
  - PE runs in 32x32 tiling mode (16 independent tiles). Tile (r, c) does
    one node's [32h x 32o] x [32h x 16b] matmul per instruction (f32
    self-loading stationary).
  - W is DMAed in its natural [node, o, h] layout (4 KB contiguous runs),
    then transposed on-chip with DVE stream-transpose (32x32 block-local)
    into [h-on-partition, (c, o, j)] layout for the PE.
  - x is DMAed as [(r, h) partitions, (b, node-run)] with long contiguous
    node runs.
  - PSUM eviction is one DVE tensor_add per round which also adds the bias
    (bias resident in SBUF, transposed once at startup, broadcast over b
    with a stride-0 AP dim).
  - Outputs are accumulated in SBUF and written back with long runs.

Node bookkeeping per core (NPAD nodes): 4 row-quadrants r own contiguous
regions of NPAD/4 nodes; within each quadrant, 4 PE columns c own contiguous
subregions of cc[c]*32 nodes (cc = [12,12,12,13] chunks for NPAD=6272).
Round t: PE tile (r, c) processes the 32 nodes of chunk t of column c.
"""

from contextlib import ExitStack

import numpy as np

import concourse.bass as bass
import concourse.mybir as mybir
import concourse.tile as tile
from concourse.ap import AP

F32 = mybir.dt.float32

B = 16  # batch
H = 32  # in channels
O = 32  # out channels
NCORES = 8
NFULL = 50000
NPC = NFULL // NCORES  # 6250 nodes per core
NPAD = 6272  # 49 * 128, per-core padded node count


def _ap(handle_ap, offset, dims):
    """Raw AP on the same tensor: dims = [(step, count), ...] in elements."""
    return AP(handle_ap.tensor, offset, [[int(s), int(c)] for s, c in dims])


def build_bass(npad=NPAD, e_rounds=4):
    """Build the per-core Bass program. npad must be a multiple of 128."""
    assert npad % 128 == 0
    q = npad // 4  # nodes per row-quadrant
    total_chunks = npad // 128  # chunks of 32 nodes per quadrant
    cc0 = total_chunks // 4  # chunks per PE column, c < 3
    cc3 = total_chunks - 3 * cc0  # chunks for column 3 (cc0..cc0+3)
    ccm = cc3  # max chunks per column
    creg = cc0 * 32  # node stride between column regions
    assert cc0 >= 1

    nc = bass.Bass()
    x_d = nc.declare_dram_parameter("x", [B, H, npad], F32, isOutput=False)
    w_d = nc.declare_dram_parameter("W", [npad, O, H], F32, isOutput=False)
    b_d = nc.declare_dram_parameter("b", [npad, O], F32, isOutput=False)
    out_d = nc.declare_dram_parameter("out", [B, O, npad], F32, isOutput=True)

    with ExitStack() as ctx:
        tc = ctx.enter_context(tile.TileContext(nc))
        xp = ctx.enter_context(tc.tile_pool(name="xp", bufs=2))
        wnp = ctx.enter_context(tc.tile_pool(name="wn", bufs=3))
        wtp = ctx.enter_context(tc.tile_pool(name="wt", bufs=2))
        outp = ctx.enter_context(tc.tile_pool(name="outp", bufs=2))
        btp = ctx.enter_context(tc.tile_pool(name="btp", bufs=1))
        bnp = ctx.enter_context(tc.tile_pool(name="bnp", bufs=2))
        psp = ctx.enter_context(tc.tile_pool(name="psp", bufs=2, space="PSUM"))

        # ---- bias: load + transpose once; resident [p=(c,o), f=(r, u)] ----
        bt = btp.tile([128, 4 * ccm * 32], F32)
        for r in range(4):
            bn = bnp.tile([128, ccm * 32], F32)
            # staging [p=(c:4, j:32), f=(ublk:ccm, o:32)]; per-ublk DMAs
            # keep each AP at 3 dims (DMA balancing limit).
            for ub in range(ccm):
                src = _ap(
                    b_d[:],
                    (r * q + ub * 32) * O,
                    [(creg * O, 4), (O, 32), (1, O)],
                )
                nc.sync.dma_start(out=bn[:, ub * 32 : (ub + 1) * 32], in_=src)
            nc.vector.transpose(bt[:, r * ccm * 32 : (r + 1) * ccm * 32], bn[:])

        n_groups = (cc0 + e_rounds - 1) // e_rounds

        for g in range(n_groups):
            t0 = g * e_rounds
            er = min(e_rounds, cc0 - t0)  # rounds in this group
            gw = er * 32  # node window per (r, c) in this group

            # ---- x for this group: 4 DMAs (one per column region) ----
            # x_t [p=(r:4, h:32), f=(c:4, b:16, m:gw)]
            x_t = xp.tile([128, 4 * B * gw], F32)
            for c in range(4):
                for bb in range(B):
                    src = _ap(
                        x_d[:],
                        bb * npad * H + c * creg + t0 * 32,
                        [(q, 4), (npad, H), (1, gw)],
                    )
                    nc.sync.dma_start(
                        out=x_t[:, (c * B + bb) * gw : (c * B + bb + 1) * gw],
                        in_=src,
                    )

            # out_t [p=(c,o), f=(b:16, r:4, w:gw)]
            out_t = outp.tile([128, B * 4 * gw], F32)

            for tl in range(er):
                t = t0 + tl
                # ---- W chunk for round t: [p=(r,j), f=(c,o,h)] ----
                # per-column staging tiles keep SBUF pressure low; each is
                # stream-transposed into its slice of wt [p=(r,h), f=(c,o,j)]
                wt = wtp.tile([128, 4096], F32)
                for c in range(4):
                    wn = wnp.tile([128, 1024], F32)
                    src = _ap(
                        w_d[:],
                        (c * creg + t * 32) * O * H,
                        [(q * O * H, 4), (O * H, 32), (H, O), (1, H)],
                    )
                    nc.sync.dma_start(out=wn[:], in_=src)
                    nc.vector.transpose(
                        wt[:, c * 1024 : (c + 1) * 1024], wn[:]
                    )

                # ---- matmuls: 16 PE tiles, 32 nodes each ----
                ps = psp.tile([128, 2048], F32)  # f=(r:4, j:32, b:16)
                wt_v = wt[:].rearrange("p (c o j) -> p c o j", c=4, o=O, j=32)
                x_v = x_t[:].rearrange("p (c b m) -> p c b m", c=4, b=B, m=gw)
                ps_v = ps[:].rearrange("p (r j b) -> p r j b", r=4, j=32, b=B)
                for j in range(32):
                    for r in range(4):
                        for c in range(4):
                            nc.tensor.matmul(
                                ps_v[32 * c : 32 * c + 32, r, j, :],
                                wt_v[32 * r : 32 * r + 32, c, :, j],
                                x_v[32 * r : 32 * r + 32, c, :, tl * 32 + j],
                                start=True,
                                stop=True,
                                tile_position=(32 * r, 32 * c),
                            )

                # ---- eviction + bias add (DVE), iter (r, j, b) ----
                out_ap = (
                    out_t[:]
                    .rearrange("p (b r w) -> p b r w", b=B, r=4, w=gw)[
                        :, :, :, tl * 32 : tl * 32 + 32
                    ]
                    .transpose([0, 2, 3, 1])
                )
                bt_ap = (
                    bt[:]
                    .rearrange("p (r u) -> p r u", r=4)[:, :, t * 32 : t * 32 + 32]
                    .unsqueeze(3)
                    .broadcast_to([128, 4, 32, B])
                )
                nc.vector.tensor_add(out_ap, ps_v[:, :, :, :], bt_ap)

            # ---- out DMAs: one per r ----
            out_v = out_t[:].rearrange("p (b r w) -> p b r w", b=B, r=4, w=gw)
            for r in range(4):
                for bb in range(B):
                    dst = _ap(
                        out_d[:],
                        bb * npad * O + r * q + t0 * 32,
                        [(creg, 4), (npad, O), (1, gw)],
                    )
                    nc.scalar.dma_start(out=dst, in_=out_v[:, bb, r, :])

        # ---- tail rounds: columns c=3 only (chunks cc0..cc3-1) ----
        for t in range(cc0, cc3):
            x3 = xp.tile([128, B * 32], F32, tag="x_t")
            for bb in range(B):
                src = _ap(
                    x_d[:],
                    bb * npad * H + 3 * creg + t * 32,
                    [(q, 4), (npad, H), (1, 32)],
                )
                nc.sync.dma_start(out=x3[:, bb * 32 : (bb + 1) * 32], in_=src)

            wn = wnp.tile([128, 1024], F32, tag="wn")
            src = _ap(
                w_d[:],
                (3 * creg + t * 32) * O * H,
                [(q * O * H, 4), (O * H, 32), (H, O), (1, H)],
            )
            nc.sync.dma_start(out=wn[:], in_=src)
            wt = wtp.tile([128, 1024], F32, tag="wt")
            nc.vector.transpose(wt[:], wn[:])

            ps = psp.tile([128, 2048], F32)
            wt_v = wt[:].rearrange("p (o j) -> p o j", o=O, j=32)
            x_v = x3[:].rearrange("p (b m) -> p b m", b=B, m=32)
            ps_v = ps[:].rearrange("p (r j b) -> p r j b", r=4, j=32, b=B)
            for j in range(32):
                for r in range(4):
                    nc.tensor.matmul(
                        ps_v[96:128, r, j, :],
                        wt_v[32 * r : 32 * r + 32, :, j],
                        x_v[32 * r : 32 * r + 32, :, j],
                        start=True,
                        stop=True,
                        tile_position=(32 * r, 96),
                    )

            out3 = outp.tile([128, 2048], F32, tag="out_t")  # f=(b,r,w:32)
            out_ap = (
                out3[96:128]
                .rearrange("p (b r w) -> p b r w", b=B, r=4, w=32)
                .transpose([0, 2, 3, 1])
            )
            bt_ap = (
                bt[96:128]
                .rearrange("p (r u) -> p r u", r=4)[:, :, t * 32 : t * 32 + 32]
                .unsqueeze(3)
                .broadcast_to([32, 4, 32, B])
            )
            nc.vector.tensor_add(out_ap, ps_v[96:128, :, :, :], bt_ap)

            out_v = out3[96:128].rearrange("p (b r w) -> p b r w", b=B, r=4, w=32)
            for r in range(4):
                dst = _ap(
                    out_d[:],
                    r * q + 3 * creg + t * 32,
                    [(npad, O), (npad * O, B), (1, 32)],
                )
                nc.scalar.dma_start(out=dst, in_=out_v[:, :, r, :])

    return nc


def _legalize_waits(nc):
    """Walrus's per-instruction sync structs carry at most one wait
    (DMA_DIRECT2D, S3_LW, ...); Tile sometimes leaves several on one
    instruction. Move the surplus onto EventSemaphore instructions inserted
    just before it on the same engine — the issuing sequencer executes its
    stream in order, so the waits still gate the instruction."""
    nsplit = 0
    for f in nc.m.functions:
        for bb in f.blocks:
            new = []
            changed = False
            for inst in bb.instructions:
                si = getattr(inst, "sync_info", None)
                if (
                    si is not None
                    and si.on_wait
                    and len(si.on_wait) > 1
                    and type(inst).__name__ != "InstEventSemaphore"
                ):
                    waits = list(si.on_wait)
                    for w in waits[:-1]:
                        nsplit += 1
                        new.append(
                            mybir.InstEventSemaphore(
                                name=f"wait-split-{nsplit}",
                                engine=inst.engine,
                                ins=[],
                                outs=[],
                                sync_info=mybir.SyncInfo(
                                    on_wait=[w], on_update=[]
                                ),
                            )
                        )
                    inst.sync_info = mybir.SyncInfo(
                        on_wait=[waits[-1]], on_update=list(si.on_update)
                    )
                    changed = True
                new.append(inst)
            if changed:
                bb.instructions = new
    return nc


_legalize_dma_waits = _legalize_waits


_NC_CACHE = {}


def _get_nc(npad=NPAD):
    if npad not in _NC_CACHE:
        _NC_CACHE[npad] = _legalize_dma_waits(build_bass(npad))
    return _NC_CACHE[npad]


def make_in_maps(x, W, b):
    x = np.ascontiguousarray(x, dtype=np.float32)
    W = np.ascontiguousarray(W, dtype=np.float32)
    b = np.ascontiguousarray(b, dtype=np.float32)
    in_maps = []
    for core in range(NCORES):
        sl = slice(core * NPC, (core + 1) * NPC)
        xs = np.zeros((B, H, NPAD), np.float32)
        xs[:, :, :NPC] = x[:, :, sl]
        Ws = np.zeros((NPAD, O, H), np.float32)
        Ws[:NPC] = W[sl]
        bs = np.zeros((NPAD, O), np.float32)
        bs[:NPC] = b[sl]
        in_maps.append({"x": xs, "W": Ws, "b": bs})
    return in_maps


def run_spmd(in_maps, **kwargs):
    from concourse.bass_utils import run_bass_kernel_spmd

    nc = _get_nc()
    return run_bass_kernel_spmd(
        nc, in_maps, core_ids=list(range(NCORES)), **kwargs
    )


def kernel(x, W, b):
    res = run_spmd(make_in_maps(x, W, b))
    out = np.concatenate(
        [res.results[c]["out"][:, :, :NPC] for c in range(NCORES)], axis=2
    )
    return out
